# revision 16
# baseline (speedup 1.0000x reference)
"""Bass/Tile kernel for nn_CNN2: lagged cross-correlation + mean/var + tiny CNN head.

Sharding: interleaved lags across 8 cores. Core m computes lags
d = 512g + 128b + 32a + 4m + (3-d1), g,b,a,d1 in [0,4).
The 4m offset is baked into per-core input data placement (pxr rows of packb);
the program is identical across cores (SPMD).

v6 (on top of v5's shift-folded W-fold + banded conv head):
- early dummy AllGather so the NRT collective rendezvous overlaps compute
- AllGather (floor ~4.6us) of a pre-symmetrized, pre-scaled [30,60] payload
  instead of AllReduce (floor ~9.7us) of [30,120]; local rank-sum on DVE
- per-core payload is a0*(A+A^T-c0*Z0) / a1*(B+B^T-c0*Z0^2), so the head
  starts directly from st1/ss equivalents (no post-collective transposes)
- bf16 conv head on 0.5-centered activations (borders become exact -0.5),
  conv1 bias plane applied as an extra accumulate matmul
- maxpool partition regroup via PE transposes instead of an SBUF->SBUF DMA
- inputs packed tightly (bf16, no zero rows) and spread over 5 DMA queues
- trace/normalize chain spread across vector/scalar/gpsimd engines
"""
import numpy as np
import ml_dtypes

import concourse.bass as bass
import concourse.bacc as bacc
import concourse.tile as tile
from concourse import mybir

bf16 = ml_dtypes.bfloat16
FP32 = mybir.dt.float32
BF16 = mybir.dt.bfloat16

T = 2048
ROW = 30
NC = 8
OL = 4            # frame offset
BM = 128          # buffer lead margin
WBUF = 2304       # 18*128
NCC = 17          # contraction chunks
NLAG = 2 * T - 1  # 4095
A0 = 0.5 / NLAG
A1 = 0.5 / (NLAG - 1)
CQ = 2.0 * NLAG / (NLAG - 1)

# packb (bf16) [64, 2336]: rows 0:30 pxl+wt, rows 32:62 pxr+wt
PB_WT = 2304
PB_COLS = 2336

# packa (bf16) [120, 64]
PA_FOLDI = 0
PA_SELI = 30
PA_BIND = 60
PA_COLS = 64

# packh (bf16) [64, ...] head constants
PH_W64 = 0          # 4 x [64, 232] banded conv1
PH_W2 = 928         # 2 x [24, 64] banded conv2
PH_ID30 = 1056      # [30, 30]
PH_ID64 = 1086      # [64, 64]
PH_GMGV = 1150      # [30, 64]; cols 0,31,32,63 = -0.5, rest runtime
PH_GPT = 1214       # [64, 32]; border rows/cols = -0.5, rest runtime
PH_B1R = 1246       # [1, 232] conv1 bias plane row
PH_ONES = 1478      # [1, 29] ones
PH_P1T5 = 1507      # [24, 5] zeros; cols 1:4 runtime
PH_FWT = 1512       # [16, 2]
PH_BINDT = 1514     # [4, 120]
PH_COLS = 1634

# packs (f32) [64, 124]
PS_BINDT = 0        # [4, 120]
PS_C0 = 120         # [30, 1]: A0 on core 0 else 0
PS_CB2 = 121        # [64, 1]
PS_FB = 122         # [1, 2]
PS_COLS = 124

INPUT_SPECS = [
    ("packb", [64, PB_COLS], BF16),
    ("diagmask", [120, 480], BF16),
    ("packa", [120, PA_COLS], BF16),
    ("packh", [64, PH_COLS], BF16),
    ("packs", [64, PS_COLS], FP32),
]


# ---------------------------------------------------------------- host prep
def host_inputs(x, W, conv1_w, conv1_b, conv2_w, conv2_b, fc_w, fc_b):
    """Returns per-core input maps (program constants + per-core data)."""
    x = np.asarray(x, np.float32)
    W = np.asarray(W, np.float32)
    conv1_w = np.asarray(conv1_w, np.float32)
    conv1_b = np.asarray(conv1_b, np.float32)
    conv2_w = np.asarray(conv2_w, np.float32)
    conv2_b = np.asarray(conv2_b, np.float32)
    fc_w = np.asarray(fc_w, np.float32)
    fc_b = np.asarray(fc_b, np.float32)

    def px(off):
        p = np.zeros((ROW, WBUF), bf16)
        w0 = BM + off
        n = min(T, WBUF - w0)
        p[:, w0:w0 + n] = x[:, :n].astype(bf16)
        return p

    wtb = np.ascontiguousarray(W.T).astype(bf16)   # [30,30]: wt[k,i] = W[i,k]
    pxl = px(OL)

    packb = np.zeros((64, PB_COLS), bf16)
    packb[0:30, 0:WBUF] = pxl
    packb[0:30, PB_WT:PB_WT + 30] = wtb
    packb[32:62, PB_WT:PB_WT + 30] = wtb

    blockind = np.zeros((120, 4), np.float32)
    diagmask = np.zeros((120, 480), np.float32)
    for d1 in range(4):
        blockind[d1 * 30:d1 * 30 + 30, d1] = 1.0
        for b2 in range(16):
            for i in range(ROW):
                diagmask[d1 * 30 + i, b2 * 30 + i] = 1.0

    packa = np.zeros((120, PA_COLS), bf16)
    for d1 in range(4):
        packa[d1 * 30:(d1 + 1) * 30, PA_FOLDI:PA_FOLDI + 30] = np.eye(30)
    packa[90:120, PA_SELI:PA_SELI + 30] = np.eye(30)
    packa[0:120, PA_BIND:PA_BIND + 4] = blockind.astype(bf16)

    packh = np.zeros((64, PH_COLS), bf16)
    for ky in range(4):
        blk = np.zeros((64, 232), np.float32)
        for ic in range(2):
            for xp in range(32):
                for xx in range(29):
                    kx = xp - xx
                    if 0 <= kx < 4:
                        for oc in range(8):
                            blk[ic * 32 + xp, oc * 29 + xx] = conv1_w[oc, ic, ky, kx]
        packh[0:64, PH_W64 + 232 * ky:PH_W64 + 232 * (ky + 1)] = blk.astype(bf16)
    for ky in range(2):
        blk = np.zeros((24, 64), np.float32)
        for c8 in range(8):
            for px_ in range(3):
                for oc2 in range(16):
                    for x2 in range(4):
                        kx = px_ - x2 + 1
                        if 0 <= kx < 2:
                            blk[c8 * 3 + px_, oc2 * 4 + x2] = conv2_w[oc2, c8, ky, kx]
        packh[0:24, PH_W2 + 64 * ky:PH_W2 + 64 * (ky + 1)] = blk.astype(bf16)
    packh[0:30, PH_ID30:PH_ID30 + 30] = np.eye(30)
    packh[0:64, PH_ID64:PH_ID64 + 64] = np.eye(64)
    gmgv = np.zeros((30, 64), np.float32)
    gmgv[:, 0] = gmgv[:, 31] = gmgv[:, 32] = gmgv[:, 63] = -0.5
    packh[0:30, PH_GMGV:PH_GMGV + 64] = gmgv.astype(bf16)
    gpt = np.zeros((64, 32), np.float32)
    gpt[0, :] = gpt[31, :] = gpt[32, :] = gpt[63, :] = -0.5
    gpt[:, 0] = gpt[:, 31] = -0.5
    packh[0:64, PH_GPT:PH_GPT + 32] = gpt.astype(bf16)
    b1p = conv1_b + 0.5 * conv1_w.sum(axis=(1, 2, 3))
    packh[0:1, PH_B1R:PH_B1R + 232] = np.repeat(b1p, 29)[None, :].astype(bf16)
    packh[0:1, PH_ONES:PH_ONES + 29] = 1.0
    packh[0:16, PH_FWT:PH_FWT + 2] = fc_w.T.astype(bf16)
    packh[0:4, PH_BINDT:PH_BINDT + 120] = blockind.T.astype(bf16)

    packs = np.zeros((64, PS_COLS), np.float32)
    packs[0:4, PS_BINDT:PS_BINDT + 120] = blockind.T
    packs[0:64, PS_CB2] = np.repeat(conv2_b, 4)
    packs[0:1, PS_FB:PS_FB + 2] = fc_b.reshape(1, 2)

    dmb = diagmask.astype(bf16)
    per_core = []
    for m in range(NC):
        pb = packb.copy()
        pb[32:62, 0:WBUF] = px(OL + 4 * m)
        ps = packs.copy()
        ps[0:30, PS_C0] = A0 if m == 0 else 0.0
        per_core.append({"packb": pb, "diagmask": dmb, "packa": packa,
                         "packh": packh, "packs": ps})
    return per_core


def mk(t, off, dims):
    """AP on tile t with explicit free dims; partition pair preserved."""
    ap = t[:]
    return bass.AP(ap.tensor, ap.offset + off,
                   [list(ap.ap[0])] + [list(d) for d in dims])


# ---------------------------------------------------------------- kernel
def build_nc():
    nc = bacc.Bacc("TRN2", target_bir_lowering=False, debug=False, num_devices=NC)
    din = {}
    for name, shape, dt in INPUT_SPECS:
        din[name] = nc.dram_tensor(name, shape, dt, kind="ExternalInput").ap()
    out_d = nc.dram_tensor("out", [1, 2], FP32, kind="ExternalOutput").ap()
    with tile.TileContext(nc) as tc:
        _body(tc, din, out_d)
    nc.compile()
    return nc


def _body(tc, din, out_d):
    nc = tc.nc
    AT = mybir.AluOpType
    AX = mybir.AxisListType
    RG = [list(range(NC))]

    from contextlib import ExitStack
    ctx = ExitStack()
    with ctx:
        consts = ctx.enter_context(tc.tile_pool(name="consts", bufs=1))
        copies_p = ctx.enter_context(tc.tile_pool(name="copies", bufs=1))
        work = ctx.enter_context(tc.tile_pool(name="work", bufs=2))
        accs_p = ctx.enter_context(tc.tile_pool(name="accs", bufs=1))
        dram = ctx.enter_context(tc.tile_pool(name="dram", bufs=1, space="DRAM"))
        headp = ctx.enter_context(tc.tile_pool(name="head", bufs=1))

        # ---- load inputs, one DMA per engine queue so they overlap
        packb = consts.tile([64, PB_COLS], BF16, tag="packb")
        nc.sync.dma_start(packb[0:32, 0:PB_COLS], din["packb"][0:32, 0:PB_COLS])
        nc.scalar.dma_start(packb[32:64, 0:PB_COLS],
                            din["packb"][32:64, 0:PB_COLS])
        diagmask = consts.tile([120, 480], BF16, tag="diagmask")
        nc.gpsimd.dma_start(diagmask[:], din["diagmask"][:])
        packa = consts.tile([120, PA_COLS], BF16, tag="packa")
        nc.gpsimd.dma_start(packa[:], din["packa"][:])
        packh = consts.tile([64, PH_COLS], BF16, tag="packh")
        nc.scalar.dma_start(packh[:], din["packh"][:])
        packs = consts.tile([64, PS_COLS], FP32, tag="packs")
        nc.scalar.dma_start(packs[:], din["packs"][:])

        wtL = packb[0:30, PB_WT:PB_WT + 30]
        wtR = packb[32:62, PB_WT:PB_WT + 30]
        foldI = packa[0:120, PA_FOLDI:PA_FOLDI + 30]
        selI = packa[0:120, PA_SELI:PA_SELI + 30]
        blockind = packa[0:120, PA_BIND:PA_BIND + 4]
        id30 = packh[0:30, PH_ID30:PH_ID30 + 30]
        id64 = packh[0:64, PH_ID64:PH_ID64 + 64]
        gmgv = packh[0:30, PH_GMGV:PH_GMGV + 64]
        gpadT = packh[0:64, PH_GPT:PH_GPT + 32]
        b1row = packh[0:1, PH_B1R:PH_B1R + 232]
        ones29 = packh[0:1, PH_ONES:PH_ONES + 29]
        ones1 = packh[0:1, PH_ONES:PH_ONES + 1]
        p1T5 = packh[0:24, PH_P1T5:PH_P1T5 + 5]
        fwt = packh[0:16, PH_FWT:PH_FWT + 2]
        bindT = packh[0:4, PH_BINDT:PH_BINDT + 120]
        c0mask = packs[0:30, PS_C0:PS_C0 + 1]
        cb2rep = packs[0:64, PS_CB2:PS_CB2 + 1]
        fb = packs[0:1, PS_FB:PS_FB + 2]

        # ---- shift-folded W-fold (see v5): psum chunks land directly in the
        # interleaved layout the main matmuls need. Chunks are cast in pairs
        # to SBUF bf16, alternating vector/scalar engines (gpsimd lacks a
        # PSUM port).
        ylc = copies_p.tile([128, NCC * 128], BF16, tag="ylc")
        yrc = copies_p.tile([128, (3 + NCC) * 120], BF16, tag="yrc")
        nc.gpsimd.memset(yrc[:, 0:360], 0.0)

        def cast_out(k, dst_ap, ps_ap):
            nc.scalar.copy(dst_ap, ps_ap)

        with tc.tile_pool(name="wfpsum", bufs=4, space="PSUM") as wf_psum:
            ylc_pairs = [(12, 13), (14, 15), (16,), (0, 1), (2, 3), (4, 5),
                         (6, 7), (8, 9), (10, 11)]
            yrc_pairs = [(3, 4), (5, 6), (7, 8), (9, 10), (11, 12), (13, 14),
                         (15, 16), (17, 18), (19,)]
            k = 0
            for pair in ylc_pairs:
                ps = wf_psum.tile([128, 120 * len(pair)], FP32, tag="wf")
                for j, c in enumerate(pair):
                    for d1 in range(4):
                        off = 128 * (c + 1) - d1
                        nc.tensor.matmul(
                            ps[:, 120 * j + 30 * d1:120 * j + 30 * d1 + 30],
                            packb[0:30, off:off + 128], wtL,
                            start=True, stop=True)
                cast_out(k, mk(ylc, 128 * pair[0],
                               [[128, len(pair)], [1, 120]]), ps[:])
                k += 1
            for pair in yrc_pairs:
                ps = wf_psum.tile([128, 120 * len(pair)], FP32, tag="wf")
                for j, cc in enumerate(pair):
                    for v in range(4):
                        off = 128 * (cc - 2) - 3 - 32 * v
                        nc.tensor.matmul(
                            ps[:, 120 * j + 30 * v:120 * j + 30 * v + 30],
                            packb[32:62, off:off + 128], wtR,
                            start=True, stop=True)
                cast_out(k, yrc[:, 120 * pair[0]:120 * (pair[-1] + 1)], ps[:])
                k += 1

        # ---- main matmul groups + trace + scaled accumulation
        accs = accs_p.tile([120, 60], FP32, tag="accs")
        z0s = accs_p.tile([ROW, ROW], FP32, tag="z0s")
        pay = accs_p.tile([ROW, 60], FP32, tag="pay")

        with tc.tile_pool(name="mmpsum", bufs=2, space="PSUM") as mm_psum, \
             tc.tile_pool(name="trpsum", bufs=2, space="PSUM") as tr_psum, \
             tc.tile_pool(name="auxpsum", bufs=1, space="PSUM") as aux_psum:
            for gi, g in enumerate((3, 2, 1, 0)):
                psf = mm_psum.tile([128, 480], FP32, tag="mm")
                for c in range(4 * g, NCC):
                    lhsT = mk(ylc, 128 * c, [[1, 128]])
                    rhs = mk(yrc, 120 * (3 + c - 4 * g), [[30, 4], [-120, 4], [1, 30]])
                    nc.tensor.matmul(psf[:], lhsT, rhs,
                                     start=(c == 4 * g), stop=(c == NCC - 1))
                ps = psf[0:120, 0:480]
                # traces: mask diag, column-fold via matmul, reduce, reciprocal
                D = work.tile([120, 480], BF16, tag="D")
                nc.vector.tensor_mul(D[:], ps, diagmask[:])
                tps = tr_psum.tile([4, 480], FP32, tag="tr")
                nc.tensor.matmul(tps[:], blockind, D[:], start=True, stop=True)
                tr = work.tile([4, 16], FP32, tag="tr16")
                nc.vector.reduce_sum(tr[:], mk(tps, 0, [[ROW, 16], [1, ROW]]),
                                     axis=AX.X)
                recip = work.tile([4, 16], BF16, tag="recip")
                with nc.allow_low_precision(reason="1/tr in bf16: 0.4% rel, "
                                            "far under the 2e-2 gate"):
                    nc.vector.reciprocal(recip[:], tr[:])
                rbp = tr_psum.tile([120, 16], FP32, tag="tr")
                nc.tensor.matmul(rbp[:], bindT, recip[:], start=True, stop=True)
                rb = work.tile([120, 16], FP32, tag="rb")
                nc.scalar.copy(rb[:], rbp[:])
                Z = work.tile([120, 480], BF16, tag="Z")
                nc.vector.tensor_mul(Z[:], ps, mk(rb, 0, [[1, 16], [0, ROW]]))
                Zsq = work.tile([120, 480], BF16, tag="Zsq")
                nc.scalar.activation(Zsq[:], Z[:],
                                     mybir.ActivationFunctionType.Square)
                zperm = mk(Z, 0, [[1, ROW], [ROW, 16]])
                zsqperm = mk(Zsq, 0, [[1, ROW], [ROW, 16]])
                if gi == 0:
                    nc.vector.reduce_sum(accs[:, 0:30], zperm, axis=AX.X)
                    nc.vector.reduce_sum(accs[:, 30:60], zsqperm, axis=AX.X)
                else:
                    t1 = work.tile([120, ROW], FP32, tag="redtmp")
                    nc.vector.reduce_sum(t1[:], zperm, axis=AX.X)
                    nc.gpsimd.tensor_add(accs[:, 0:30], accs[:, 0:30], t1[:])
                    t2 = work.tile([120, ROW], FP32, tag="redtmp2")
                    nc.vector.reduce_sum(t2[:], zsqperm, axis=AX.X)
                    nc.gpsimd.tensor_add(accs[:, 30:60], accs[:, 30:60], t2[:])
                if g == 0:
                    z0p = aux_psum.tile([ROW, ROW], FP32, tag="aux")
                    nc.tensor.matmul(z0p[:], selI, Z[:, 0:30],
                                     start=True, stop=True)
                    nc.scalar.copy(z0s[:], z0p[:])

            # ---- payload: a0*(A + A^T - c0 Z0) | a1*(B + B^T - c0 Z0^2)
            accsb = accs_p.tile([120, 60], BF16, tag="accsb")
            nc.scalar.mul(accsb[:, 0:30], accs[:, 0:30], A0)
            nc.scalar.mul(accsb[:, 30:60], accs[:, 30:60], A1)
            foldps = aux_psum.tile([ROW, 60], FP32, tag="aux")
            nc.tensor.matmul(foldps[:, 0:30], foldI, accsb[:, 0:30],
                             start=True, stop=False)
            nc.tensor.matmul(foldps[:, 0:30], accsb[:, 0:30], foldI,
                             start=False, stop=True)
            nc.tensor.matmul(foldps[:, 30:60], foldI, accsb[:, 30:60],
                             start=True, stop=False)
            nc.tensor.matmul(foldps[:, 30:60], accsb[:, 30:60], foldI,
                             start=False, stop=True)
            zc0 = headp.tile([ROW, ROW], FP32, tag="zc0")
            nc.vector.tensor_scalar_mul(zc0[:], z0s[:], c0mask)
            nc.vector.tensor_sub(pay[:, 0:30], foldps[:, 0:30], zc0[:])
            t30 = headp.tile([ROW, ROW], FP32, tag="t30")
            nc.vector.scalar_tensor_tensor(t30[:], zc0[:], A1 / A0, z0s[:],
                                           op0=AT.mult, op1=AT.mult)
            nc.vector.tensor_sub(pay[:, 30:60], foldps[:, 30:60], t30[:])

        # ---- AllGather + local rank-sum
        cc_in = dram.tile([ROW, 60], FP32, tag="ccin")
        cc_out = dram.tile([NC * ROW, 60], FP32, tag="ccout")
        nc.sync.dma_start(cc_in[:], pay[:])
        nc.gpsimd.collective_compute(
            "AllGather", AT.bypass, replica_groups=RG,
            ins=[cc_in.opt()], outs=[cc_out.opt()])
        res8 = accs_p.tile([ROW, 8 * 60], FP32, tag="res8")
        co = cc_out[:]
        nc.sync.dma_start(res8[:], bass.AP(
            co.tensor, co.offset, [[60, 30], [30 * 60, 8], [1, 60]]))
        res = accs_p.tile([ROW, 60], FP32, tag="res")
        nc.vector.reduce_sum(res[:], mk(res8, 0, [[1, 60], [60, 8]]),
                             axis=AX.X)

        # ---- head: res[:,0:30] = am (a0*st1), res[:,30:60] = a1*ss
        with tc.tile_pool(name="headpsum", bufs=2, space="PSUM") as head_psum:
            q0 = headp.tile([ROW, ROW], FP32, tag="q0")
            nc.gpsimd.tensor_mul(q0[:], res[:, 0:30], res[:, 0:30])
            nc.vector.tensor_copy(gmgv[:, 1:31], res[:, 0:30])
            nc.vector.scalar_tensor_tensor(gmgv[:, 33:63], q0[:], -CQ,
                                           res[:, 30:60], op0=AT.mult, op1=AT.add)
            gtp = head_psum.tile([64, ROW], FP32, tag="hps")
            nc.tensor.matmul(gtp[:], gmgv, id30, start=True, stop=True)
            nc.vector.tensor_copy(gpadT[:, 1:31], gtp[:])

            # conv1: bias plane + 4 banded matmuls, all bf16
            h1p = head_psum.tile([29, 232], FP32, tag="hps")
            nc.tensor.matmul(h1p[:], ones29, b1row, start=True, stop=False)
            for ky in range(4):
                nc.tensor.matmul(
                    h1p[:], gpadT[:, ky:ky + 29],
                    packh[0:64, PH_W64 + 232 * ky:PH_W64 + 232 * (ky + 1)],
                    start=False, stop=(ky == 3))
            h1s = headp.tile([29, 232], BF16, tag="h1s")
            nc.scalar.activation(h1s[:], h1p[:],
                                 mybir.ActivationFunctionType.Prelu, alpha=0.2)
            # maxpool 8x8: pool-x (free dim), transpose, pool-y
            px1 = headp.tile([29, 24], BF16, tag="px1")
            nc.vector.reduce_max(px1[:], mk(h1s, 0, [[29, 8], [8, 3], [1, 8]]),
                                 axis=AX.X)
            t1ps = head_psum.tile([24, 29], FP32, tag="hps")
            nc.tensor.matmul(t1ps[:], px1[:],
                             packh[0:29, PH_ID30:PH_ID30 + 29],
                             start=True, stop=True)
            nc.vector.reduce_max(p1T5[:, 1:4], mk(t1ps, 0, [[8, 3], [1, 8]]),
                                 axis=AX.X)
            # conv2 as 2 banded matmuls: h2p[(oc2,x2), y2] (64, 4)
            h2p = head_psum.tile([64, 4], FP32, tag="hps")
            for ky in range(2):
                nc.tensor.matmul(
                    h2p[:], packh[0:24, PH_W2 + 64 * ky:PH_W2 + 64 * (ky + 1)],
                    p1T5[:, ky:ky + 4], start=(ky == 0), stop=(ky == 1))
            h2s = headp.tile([64, 4], FP32, tag="h2s")
            nc.scalar.activation(h2s[:], h2p[:],
                                 mybir.ActivationFunctionType.Prelu,
                                 bias=cb2rep, alpha=0.2)
            # maxpool 4x4 (global): reduce y2, PE transposes for the
            # cross-partition max (no SBUF->SBUF DMA)
            h2r = headp.tile([64, 1], BF16, tag="h2r")
            nc.vector.reduce_max(h2r[:], h2s[:], axis=AX.X)
            t8 = head_psum.tile([1, 64], FP32, tag="hps")
            nc.tensor.matmul(t8[:], h2r[:], id64, start=True, stop=True)
            h3r = headp.tile([1, 16], BF16, tag="h3r")
            nc.vector.reduce_max(h3r[:], mk(t8, 0, [[4, 16], [1, 4]]),
                                 axis=AX.X)
            h3c = head_psum.tile([16, 1], FP32, tag="hps")
            nc.tensor.matmul(h3c[:], h3r[:], ones1, start=True, stop=True)
            h3b = headp.tile([16, 1], BF16, tag="h3b")
            nc.vector.tensor_copy(h3b[:], h3c[:])
            fcp = head_psum.tile([1, 2], FP32, tag="hps")
            nc.tensor.matmul(fcp[:], h3b[:], fwt, start=True, stop=True)
            osb = headp.tile([1, 2], FP32, tag="osb")
            nc.vector.tensor_add(osb[:], fcp[:], fb)
            nc.sync.dma_start(out_d[:], osb[:])


# ---------------------------------------------------------------- entrypoint
_NC_CACHE = []


def kernel(**inputs):
    """Full inputs -> full output (1,2) float32. Shards internally across 8 cores."""
    from concourse.bass_utils import run_bass_kernel_spmd
    if not _NC_CACHE:
        _NC_CACHE.append(build_nc())
    nc = _NC_CACHE[0]
    maps = host_inputs(**{k: np.asarray(v) for k, v in inputs.items()})
    res = run_bass_kernel_spmd(nc, maps, core_ids=list(range(NC)))
    return np.asarray(res.results[0]["out"], np.float32)
